# revision 15
# baseline (speedup 1.0000x reference)
"""Trainium2 Bass kernel for nn_LocalAttention (depthwise causal conv + RoPE +
windowed local attention), data-parallel over the batch dim on 8 NeuronCores.

Self-contained: hardcodes shapes B=32, N=4096, D=64, WS=128 and the sharding
(4 batches per core). Host-side prep is limited to dtype casts and layout
transforms; all FLOPs over the activations run on device.

Key layout/schedule choices:
  - conv weights ship as per-channel CIRCULANT matrices C[j,i] = w[(j-i-1)%128]
    (half the bytes of separate hi/lo Toeplitz bands, since hi+lo == C with
    disjoint supports). On-chip, lo = strict-lower(C) via affine_select, and
    the causal windowed conv is  out_w = C^T x_w + lo^T (x_{w-1} - x_w).
  - conv runs per 8-channel PSUM group; x/diff/C/lo chunk tiles rotate.
  - q/k transposes to head-dim-major are identity matmuls that pack both
    batches of a pair into one 128-col stationary load (1 matmul per window).
  - sim matmuls per batch pair run as two concurrent 64-row tiles; softmax
    denominator comes from a ones-column appended to V; causal masking is a
    post-exp triangular multiply (DVE for pair 0, GpSimd for pair 1).
  - emission order keeps the PE dense: conv q -> conv k -> transposes ->
    sim pair0 -> v-conv (overlapping exp pair0 on ACT) -> sim pair1 ->
    AV pair0 -> AV pair1, with v inputs DMA'd last.
"""

import sys

sys.path.insert(0, "/opt/trn_rl_repo")

import ml_dtypes
import numpy as np

import concourse.bass as bass
import concourse.mybir as mybir
import concourse.tile as tile
from concourse.bass_utils import run_bass_kernel_spmd
from concourse.masks import make_identity

BF16 = mybir.dt.bfloat16
F32 = mybir.dt.float32
NPBF = ml_dtypes.bfloat16

B, N, D, WS = 32, 4096, 64, 128
W = N // WS              # 32 windows
NCORES = 8
BL = B // NCORES         # 4 batches per core
SCALE = D ** -0.5
ROPE_BASE = 10000.0

XCOLS = D * W * BL       # x / qc / kc cols: (d, w, b) = 8192
VCOLS = (D + 1) * W * BL  # vc cols: (d(+ones), w, b) = 8320
PCOLS = 2 * W * 2 * WS   # p cols per pair: (b2, m, 256) = 16384
PB = W * 2 * WS          # 8192: p cols per batch
CH_D = 16                # channels per input chunk
CH_COLS = CH_D * WS      # circulant-chunk cols (d, i) = 2048
XCH_COLS = CH_D * W * BL  # x-chunk cols (d, w, b) = 2048
NCH = D // CH_D          # 4 chunks per tensor


def _split_multiwaits(nc, max_waits=1):
    """walrus in this env rejects >1 sem wait per instruction; split extras
    into standalone NoOp waits inserted just before, on the same engine."""
    n_fixed = 0
    for fn in nc.m.functions:
        for bb in fn.blocks:
            insts = bb.instructions
            new_list = []
            changed = False
            for inst in insts:
                si = inst.sync_info
                if si is not None and si.on_wait and len(si.on_wait) > max_waits:
                    waits = list(si.on_wait)
                    for w in waits[:-max_waits]:
                        nop = mybir.InstNoOp(
                            name=f"{inst.name}-xw{n_fixed}",
                            engine=inst.engine,
                            ins=[],
                            outs=[],
                            sync_info=mybir.SyncInfo(on_wait=[w], on_update=[]),
                        )
                        new_list.append(nop)
                        n_fixed += 1
                    si.on_wait = waits[-max_waits:]
                    changed = True
                new_list.append(inst)
            if changed:
                bb.instructions = new_list
    return n_fixed


def _ap(t, offset, dims):
    """AP over tile t: partition dim kept, free dims replaced."""
    return bass.AP(tensor=t.tensor, offset=t.offset + offset, ap=[t.ap[0]] + dims)


def _ap_dram(t, offset, dims, row_elems):
    """AP over a [128, row_elems] dram tensor: partition stride row_elems."""
    return bass.AP(tensor=t, offset=offset, ap=[[row_elems, 128]] + dims)


def _build_program():
    nc = bass.Bass()
    # x: [128 j, (d, w, b)] host-pretransposed, fully contiguous
    xq = nc.dram_tensor("xq", [128, XCOLS], BF16, kind="ExternalInput")
    xk = nc.dram_tensor("xk", [128, XCOLS], BF16, kind="ExternalInput")
    xv = nc.dram_tensor("xv", [128, XCOLS], BF16, kind="ExternalInput")
    # circulant conv weights: [128 j, (d, i)], C[j,i] = w[(j-i-1)%128]
    cq = nc.dram_tensor("cq", [128, D * WS], BF16, kind="ExternalInput")
    ck = nc.dram_tensor("ck", [128, D * WS], BF16, kind="ExternalInput")
    cv = nc.dram_tensor("cv", [128, D * WS], BF16, kind="ExternalInput")
    # rope tables: [128 i, (d, w, b2)] (replicated over the 2 pair batches)
    cosb = nc.dram_tensor("cosb", [128, D * W * 2], BF16, kind="ExternalInput")
    sinb = nc.dram_tensor("sinb", [128, D * W * 2], BF16, kind="ExternalInput")
    out = nc.dram_tensor("out", [BL, N, D], F32, kind="ExternalOutput")

    xdram = {"q": xq, "k": xk, "v": xv}
    cdram = {"q": cq, "k": ck, "v": cv}

    with tile.TileContext(nc) as tc:
        import contextlib

        with contextlib.ExitStack() as ctx:
            const = ctx.enter_context(tc.tile_pool(name="const", bufs=1))
            tabs = ctx.enter_context(tc.tile_pool(name="tabs", bufs=1))
            cpool = ctx.enter_context(tc.tile_pool(name="cw", bufs=5))
            xpool = ctx.enter_context(tc.tile_pool(name="x", bufs=5))
            lpool = ctx.enter_context(tc.tile_pool(name="lo", bufs=3))
            dpool = ctx.enter_context(tc.tile_pool(name="df", bufs=3))
            rtmp = ctx.enter_context(tc.tile_pool(name="rt", bufs=1))
            big = ctx.enter_context(tc.tile_pool(name="big", bufs=2))
            vpool = ctx.enter_context(tc.tile_pool(name="v", bufs=1))
            qtp = ctx.enter_context(tc.tile_pool(name="qt", bufs=4))
            opool = ctx.enter_context(tc.tile_pool(name="o", bufs=2))
            spool = ctx.enter_context(tc.tile_pool(name="s", bufs=2))

            # ---- input DMAs, earliest-needed first, split across the two
            # HWDGE queues (sync + activation engine).
            costab = tabs.tile([128, D * W * 2], BF16)
            sintab = tabs.tile([128, D * W * 2], BF16)

            cch = {}
            xch = {}
            rr = [0]

            def q_alt():
                rr[0] += 1
                return nc.sync if rr[0] % 2 else nc.scalar

            def issue_inputs(name):
                for c in range(NCH):
                    ct = cpool.tile([128, CH_COLS], BF16, tag="cw",
                                    name=f"c_{name}{c}")
                    cch[(name, c)] = ct
                    q_alt().dma_start(
                        out=ct[:],
                        in_=_ap_dram(cdram[name], c * CH_COLS, [[1, CH_COLS]],
                                     D * WS))
                    xt = xpool.tile([128, XCH_COLS], BF16, tag="x",
                                    name=f"x_{name}{c}")
                    xch[(name, c)] = xt
                    q_alt().dma_start(
                        out=xt[:],
                        in_=_ap_dram(xdram[name], c * XCH_COLS, [[1, XCH_COLS]],
                                     XCOLS))

            issue_inputs("q")
            # rope tables are not needed until rope-q (~after conv q)
            nc.sync.dma_start(
                out=costab[:], in_=_ap_dram(cosb, 0, [[1, D * W * 2]], D * W * 2))
            nc.scalar.dma_start(
                out=sintab[:], in_=_ap_dram(sinb, 0, [[1, D * W * 2]], D * W * 2))
            issue_inputs("k")
            issue_inputs("v")

            # constants
            ident = const.tile([128, 128], BF16)
            make_identity(nc, ident)
            tri = const.tile([128, 128], BF16)  # tri[j,i] = 1 if i>=j else 0
            nc.vector.memset(tri[:], 1.0)
            nc.gpsimd.affine_select(
                out=tri[:], in_=tri[:], compare_op=mybir.AluOpType.is_ge,
                fill=0.0, base=0, channel_multiplier=-1, pattern=[[1, 128]],
            )
            tri2 = const.tile([128, 128], BF16)  # tri2[j,i] = 1 if i<j else 0
            nc.vector.memset(tri2[:], 1.0)
            nc.gpsimd.affine_select(
                out=tri2[:], in_=tri2[:], compare_op=mybir.AluOpType.is_ge,
                fill=0.0, base=-1, channel_multiplier=1, pattern=[[-1, 128]],
            )

            # persistent activation tiles; qc+kc share one big buffer, p gets
            # a fresh one per pair (rotating back onto qc/kc space for pair 1)
            qckc = big.tile([128, 2 * XCOLS], BF16, tag="big", name="qckc")
            vc = vpool.tile([128, VCOLS], BF16)            # (d|ones, w, b)
            nc.vector.memset(vc[:, D * BL * W:], 1.0)      # ones column block

            drain_rr = [0]

            def drain(dst, src):
                # GpSimd cannot access PSUM; weight 2/3 ACT, 1/3 DVE
                e = drain_rr[0] % 3
                drain_rr[0] += 1
                if e != 1:
                    nc.scalar.copy(dst, src)
                else:
                    nc.vector.tensor_copy(dst, src)

            def emit_chunk_prep(name, c):
                """lo = strict-lower(C) on DVE; diff = x_{w-1}-x_w on GpSimd
                (diff[w=0] = -x_0 so the lo-matmul needs no window offset)."""
                ct = cch[(name, c)]
                lo = lpool.tile([128, CH_COLS], BF16, tag="lo",
                                name=f"lo{name}{c}")
                nc.gpsimd.tensor_mul(
                    lo[:], ct[:], _ap(tri2, 0, [[0, CH_D], [1, WS]]))
                xt = xch[(name, c)]
                df = dpool.tile([128, XCH_COLS], BF16, tag="df",
                                name=f"df{name}{c}")
                nc.vector.tensor_sub(
                    _ap(df, BL, [[W * BL, CH_D], [1, (W - 1) * BL]]),
                    _ap(xt, 0, [[W * BL, CH_D], [1, (W - 1) * BL]]),
                    _ap(xt, BL, [[W * BL, CH_D], [1, (W - 1) * BL]]),
                )
                nc.vector.tensor_scalar_mul(
                    _ap(df, 0, [[W * BL, CH_D], [1, BL]]),
                    _ap(xt, 0, [[W * BL, CH_D], [1, BL]]),
                    -1.0,
                )
                return lo, df

            def emit_rope(base, pair, eng):
                """in-place rope on (w, b2, d) views of qckc at col base."""
                po = base + pair * 2 * D  # col offset of batch pair
                t2 = rtmp.tile([128, W * 2 * D], BF16, tag="rt",
                               name=f"rt{pair}")
                # t2[d] = x[d^1] * sin[d]  (sin pre-signed by output parity);
                # partner swap via inner (u, e) dims reading col 2u + (1-e)
                part_in = _ap(qckc, po + 1,
                              [[4 * D, W], [D, 2], [2, D // 2], [-1, 2]])
                sin_in = _ap(sintab, 0,
                             [[2 * D, W], [D, 2], [2, D // 2], [1, 2]])
                t2v = _ap(t2, 0, [[2 * D, W], [D, 2], [2, D // 2], [1, 2]])
                eng.tensor_mul(t2v, part_in, sin_in)
                xv_ = _ap(qckc, po, [[4 * D, W], [1, 2 * D]])
                cos_in = _ap(costab, 0, [[2 * D, W], [1, 2 * D]])
                eng.tensor_mul(xv_, xv_, cos_in)
                eng.tensor_add(xv_, xv_, _ap(t2, 0, [[2 * D, W], [1, 2 * D]]))

            qT = {}  # (tensor, pair) -> [128 (b2,d), (w,i)] bf16

            def emit_transposes(name, pair, pool):
                base = 0 if name == "q" else XCOLS
                qt = qtp.tile([128, W * WS], BF16, tag="qt",
                              name=f"qt_{name}{pair}")
                for w4 in range(8):
                    tp = pool.tile([128, 512], F32, name=f"tp{pair}")
                    for wi in range(4):
                        w = w4 * 4 + wi
                        # stationary = (b2, d) cols of window w: contiguous
                        co = base + w * BL * D + pair * 2 * D
                        src = qckc[:, co: co + 2 * D]
                        nc.tensor.matmul(
                            tp[:, wi * 128: (wi + 1) * 128],
                            src, ident[:], start=True, stop=True,
                        )
                    drain(qt[:, w4 * 512: (w4 + 1) * 512], tp[:])
                qT[(name, pair)] = qt

            # ---- conv q, k via circulant + lower-band matmuls
            with tc.tile_pool(name="convps", bufs=2, space="PSUM") as convps, \
                 tc.tile_pool(name="tps", bufs=2, space="PSUM") as tps:
                def emit_conv_qk(name, c, lo, df):
                    base = 0 if name == "q" else XCOLS
                    ct = cch[(name, c)]
                    xt = xch[(name, c)]
                    for g2 in range(2):  # 8 channels per psum group
                        # psum layout (h=dd//4, w, b, dd%4): each matmul's
                        # strided output stays inside one bank
                        cp = convps.tile([128, 8 * 128], F32)
                        for dd in range(8):
                            dl = g2 * 8 + dd   # channel within chunk
                            po = (dd // 4) * 512 + dd % 4
                            nc.tensor.matmul(
                                _ap(cp, po, [[16, W], [4, BL]]),
                                ct[:, dl * WS: (dl + 1) * WS],
                                _ap(xt, dl * W * BL, [[1, W * BL]]),
                                start=True, stop=False)
                            nc.tensor.matmul(
                                _ap(cp, po, [[16, W], [4, BL]]),
                                lo[:, dl * WS: (dl + 1) * WS],
                                _ap(df, dl * W * BL, [[1, W * BL]]),
                                start=False, stop=True,
                                skip_group_check=True)
                        d0 = base + (c * 2 + g2) * 8
                        dst = _ap(qckc, d0,
                                  [[4, 2], [BL * D, W], [D, BL], [1, 4]])
                        drain(dst, cp[:])

                for c in range(NCH):
                    lo, df = emit_chunk_prep("q", c)
                    emit_conv_qk("q", c, lo, df)
                lo, df = emit_chunk_prep("k", 0)
                emit_conv_qk("k", 0, lo, df)
                emit_rope(0, 0, nc.vector)
                lo, df = emit_chunk_prep("k", 1)
                emit_conv_qk("k", 1, lo, df)
                emit_rope(0, 1, nc.vector)
                lo, df = emit_chunk_prep("k", 2)
                emit_conv_qk("k", 2, lo, df)
                lo, df = emit_chunk_prep("k", 3)
                emit_conv_qk("k", 3, lo, df)
                emit_rope(XCOLS, 0, nc.vector)

                # ---- transposes: q transposes overlap rope-k on DVE;
                # pair 0 of both tensors first so sim can start early
                emit_transposes("q", 0, tps)
                emit_rope(XCOLS, 1, nc.vector)
                emit_transposes("k", 0, tps)
                emit_transposes("q", 1, tps)
                emit_transposes("k", 1, tps)

            # ---- attention + v-conv
            with tc.tile_pool(name="convps2", bufs=2, space="PSUM") as convps2, \
                 tc.tile_pool(name="simps", bufs=2, space="PSUM") as simps, \
                 tc.tile_pool(name="avps", bufs=2, space="PSUM") as avps:

                pt = {}

                def emit_sim(pair):
                    qt = qT[("q", pair)]
                    kt = qT[("k", pair)]
                    p = big.tile([128, PCOLS], BF16, tag="big",
                                 name=f"p{pair}")
                    pt[pair] = p
                    # sim passes, 2 m's per group, both batches via row tiles
                    for g2 in range(16):
                        sp = simps.tile([128, 1024], F32)
                        m0 = g2 * 2
                        ncol_g = 0
                        for mi in range(2):
                            m = m0 + mi
                            ncols = 256 if m < W - 1 else 128
                            for h in range(2):  # batch half (row tile)
                                nc.tensor.matmul(
                                    sp[:, h * 512 + mi * 256:
                                       h * 512 + mi * 256 + ncols],
                                    kt[h * 64: h * 64 + 64,
                                       m * 128: (m + 1) * 128],
                                    qt[h * 64: h * 64 + 64,
                                       m * 128: m * 128 + ncols],
                                    start=True, stop=True,
                                )
                            ncol_g += ncols
                        # exp of both banks -> p[(b2, m0..m0+1, :)]
                        esrc = _ap(sp, 0, [[512, 2], [1, ncol_g]])
                        edst = _ap(p, m0 * 256, [[PB, 2], [1, ncol_g]])
                        nc.scalar.activation(
                            edst, esrc, mybir.ActivationFunctionType.Exp)

                def emit_mask(pair):
                    p = pt[pair]
                    # pad-row fixup (global key position 0 fully masked)
                    for b2 in range(2):
                        nc.vector.memset(p[0:1, b2 * PB: b2 * PB + 256], 0.0)
                    # causal mask: own-halves *= tri; per 4-m chunk,
                    # alternating DVE / GpSimd
                    for mc in range(8):
                        pview = _ap(p, mc * 4 * 256,
                                    [[PB, 2], [256, 4], [1, 128]])
                        tri_b = _ap(tri, 0, [[0, 2], [0, 4], [1, 128]])
                        eng = nc.gpsimd if mc % 2 else nc.vector
                        eng.tensor_mul(pview, pview, tri_b)
                    # all-masked-row fixup: query 0 attends uniformly
                    for b2 in range(2):
                        nc.vector.memset(p[:, b2 * PB: b2 * PB + 1], 1.0)

                def emit_vconv_part(part):
                    # v-conv in 4-channel psum groups (flat: vc is (d, w, b))
                    for c in range(2 * part, 2 * part + 2):
                        lo, df = emit_chunk_prep("v", c)
                        ct = cch[("v", c)]
                        xt = xch[("v", c)]
                        for g4 in range(4):  # 4 channels per psum group
                            cp = convps2.tile([128, 4 * 128], F32,
                                              name="vcp")
                            for dd in range(4):
                                dl = g4 * 4 + dd
                                ps = cp[:, dd * 128: (dd + 1) * 128]
                                nc.tensor.matmul(
                                    ps,
                                    ct[:, dl * WS: (dl + 1) * WS],
                                    _ap(xt, dl * W * BL, [[1, W * BL]]),
                                    start=True, stop=False)
                                nc.tensor.matmul(
                                    ps,
                                    lo[:, dl * WS: (dl + 1) * WS],
                                    _ap(df, dl * W * BL, [[1, W * BL]]),
                                    start=False, stop=True,
                                    skip_group_check=True)
                            dst = vc[:, (c * 4 + g4) * 512:
                                     (c * 4 + g4 + 1) * 512]
                            nc.vector.tensor_copy(dst, cp[:])

                def emit_av(pair, chunk, b2):
                    p = pt[pair]
                    b = pair * 2 + b2
                    pb = b2 * PB
                    sr = spool.tile([128, 8], F32, tag="sr")
                    if True:
                        w0 = chunk * 7
                        nwin = min(7, W - w0)
                        ot = opool.tile([128, 7 * D], F32, tag="ot")
                        av = avps.tile([128, 512], F32)
                        for k in range(nwin):
                            w = w0 + k
                            own = p[:, pb + w * 256: pb + w * 256 + 128]
                            ov = av[:, k * 65: k * 65 + 65]
                            vw = _ap(vc, w * BL + b, [[W * BL, D + 1]])
                            first_only = w == 0
                            nc.tensor.matmul(ov, own, vw, start=True,
                                             stop=first_only)
                            if w > 0:
                                prev = p[:, pb + (w - 1) * 256 + 128:
                                         pb + w * 256]
                                vprev = _ap(vc, (w - 1) * BL + b,
                                            [[W * BL, D + 1]])
                                nc.tensor.matmul(ov, prev, vprev,
                                                 start=False, stop=True,
                                                 skip_group_check=True)
                        if chunk == 0:
                            # window-0 query-0 sum correction (+128 pad)
                            nc.vector.tensor_scalar_add(
                                av[0:1, 64:65], av[0:1, 64:65], 128.0)
                        # normalize: recip of s, broadcast-mul
                        nc.vector.reciprocal(
                            sr[:, :nwin], _ap(av, 64, [[65, nwin]]))
                        avv = _ap(av, 0, [[65, nwin], [1, D]])
                        srv = _ap(sr, 0, [[1, nwin], [0, D]])
                        otv = _ap(ot, 0, [[D, nwin], [1, D]])
                        nc.vector.tensor_mul(otv, avv, srv)
                        dstd = bass.AP(
                            tensor=out, offset=b * N * D + w0 * WS * D,
                            ap=[[D, 128], [WS * D, nwin], [1, D]],
                        )
                        oeng = nc.scalar if (pair == 1 and b2 == 1) else nc.sync
                        oeng.dma_start(
                            out=dstd, in_=_ap(ot, 0, [[D, nwin], [1, D]]))

                emit_sim(0)
                emit_vconv_part(0)
                emit_vconv_part(1)
                emit_sim(1)
                emit_mask(0)
                for chunk in range(5):
                    emit_av(0, chunk, 0)
                    emit_av(0, chunk, 1)
                emit_mask(1)
                for chunk in range(5):
                    emit_av(1, chunk, 0)
                    emit_av(1, chunk, 1)

    _split_multiwaits(nc)
    return nc


_PROG = None


def _get_prog():
    global _PROG
    if _PROG is None:
        _PROG = _build_program()
    return _PROG


def _host_prep(q, k, v, wq, wk, wv):
    """Build per-core input maps (bf16 casts + layout transforms)."""
    jj = np.arange(WS)[:, None]
    ii = np.arange(WS)[None, :]
    cidx = (jj - ii - 1) % WS    # circulant: C[j,i] = w[(j-i-1)%WS]

    def circ(w, scale=1.0):
        wd = np.asarray(w, np.float32).reshape(D, WS) * scale
        t = wd[:, cidx]                       # [d, j, i]
        return np.ascontiguousarray(t.transpose(1, 0, 2)).reshape(
            WS, D * WS).astype(NPBF)

    cq_np = circ(wq, SCALE)
    ck_np = circ(wk)
    cv_np = circ(wv)

    theta = 1.0 / ROPE_BASE ** (np.arange(0, D, 2, dtype=np.float32) / D)
    pm = np.arange(N, dtype=np.float32)[:, None] * theta[None, :]
    cos = np.repeat(np.cos(pm), 2, axis=-1)  # [n, d]
    sin = np.repeat(np.sin(pm), 2, axis=-1)
    sgn = np.where(np.arange(D) % 2 == 0, -1.0, 1.0).astype(np.float32)

    # [j, (w, b2, d)] layout, replicated over the 2 batches of a pair
    def rope_table(t):
        tt = np.ascontiguousarray(t.reshape(W, WS, D).transpose(1, 0, 2))
        tt = np.repeat(tt.reshape(WS, W, 1, D), 2, axis=2)
        return np.ascontiguousarray(tt).reshape(WS, W * 2 * D).astype(NPBF)

    cosb_np = rope_table(cos)
    sinb_np = rope_table(sin * sgn[None, :])

    def xprep(x, sl):
        # [BL, N, D] -> [j, (d, w, b)] contiguous bf16
        xb = np.asarray(x[sl], np.float32).reshape(BL, W, WS, D)
        return np.ascontiguousarray(xb.transpose(2, 3, 1, 0)).reshape(
            WS, D * W * BL).astype(NPBF)

    in_maps = []
    for c in range(NCORES):
        sl = slice(c * BL, (c + 1) * BL)
        in_maps.append({
            "xq": xprep(q, sl),
            "xk": xprep(k, sl),
            "xv": xprep(v, sl),
            "cq": cq_np, "ck": ck_np, "cv": cv_np,
            "cosb": cosb_np, "sinb": sinb_np,
        })
    return in_maps


def _install_ntff_hook():
    """Provide antenv.axon_hooks with a ctypes NTFF profile hook (the slim
    container lacks it); enables trace=True under axon."""
    import sys as _sys
    import types
    import ctypes
    import contextlib

    try:
        from antenv.axon_hooks import get_axon_ntff_profile_hook  # noqa: F401
        return
    except ImportError:
        pass
    so_path = "/opt/axon/libaxon_pjrt.so"
    try:
        lib = ctypes.CDLL(so_path)
    except OSError:
        return
    if not hasattr(lib, "axon_start_nrt_profile"):
        return
    lib.axon_start_nrt_profile.argtypes = [
        ctypes.POINTER(ctypes.c_int64), ctypes.c_size_t]
    lib.axon_start_nrt_profile.restype = ctypes.c_int64
    lib.axon_stop_nrt_profile.argtypes = [ctypes.c_char_p]
    lib.axon_stop_nrt_profile.restype = ctypes.c_int64

    @contextlib.contextmanager
    def _hook(output_dir, device_ids):
        import jax
        jax.devices()
        if device_ids:
            ids = (ctypes.c_int64 * len(device_ids))(*device_ids)
            rc = lib.axon_start_nrt_profile(ids, len(device_ids))
        else:
            rc = lib.axon_start_nrt_profile(None, 0)
        if rc != 0:
            raise RuntimeError(f"axon_start_nrt_profile rc={rc}")
        try:
            yield
        finally:
            n = lib.axon_stop_nrt_profile(str(output_dir).encode())
            print(f"profile: {n} file(s) written to {output_dir}")

    import antenv

    mod = types.ModuleType("antenv.axon_hooks")
    _state = {"hook": _hook}
    mod.set_axon_ntff_profile_hook = lambda h: _state.__setitem__("hook", h)
    mod.get_axon_ntff_profile_hook = lambda: _state["hook"]
    _sys.modules["antenv.axon_hooks"] = mod
    antenv.axon_hooks = mod


def run(q, k, v, wq, wk, wv, trace=False):
    nc = _get_prog()
    in_maps = _host_prep(q, k, v, wq, wk, wv)
    if trace:
        _install_ntff_hook()
    res = run_bass_kernel_spmd(nc, in_maps, core_ids=list(range(NCORES)),
                               trace=trace)
    outp = np.concatenate([res.results[c]["out"] for c in range(NCORES)], axis=0)
    return outp, res


def kernel(q, k, v, wq, wk, wv):
    outp, _ = run(q, k, v, wq, wk, wv)
    return outp


# revision 16
# speedup vs baseline: 1.4124x; 1.4124x over previous
"""Trainium2 Bass kernel for nn_LocalAttention (depthwise causal conv + RoPE +
windowed local attention), data-parallel over the batch dim on 8 NeuronCores.

Self-contained: hardcodes shapes B=32, N=4096, D=64, WS=128 and the sharding
(4 batches per core). Host-side prep is limited to dtype casts and layout
transforms; all FLOPs over the activations run on device.

Key layout/schedule choices:
  - conv weights ship as per-channel CIRCULANT matrices C[j,i] = w[(j-i-1)%128]
    (half the bytes of separate hi/lo Toeplitz bands, since hi+lo == C with
    disjoint supports). On-chip, lo = strict-lower(C) via affine_select, and
    the causal windowed conv is  out_w = C^T x_w + lo^T (x_{w-1} - x_w).
  - conv runs per 8-channel PSUM group; x/diff/C/lo chunk tiles rotate.
  - q/k transposes to head-dim-major are identity matmuls that pack both
    batches of a pair into one 128-col stationary load (1 matmul per window).
  - sim matmuls per batch pair run as two concurrent 64-row tiles; softmax
    denominator comes from a ones-column appended to V; causal masking is a
    post-exp triangular multiply (DVE for pair 0, GpSimd for pair 1).
  - emission order keeps the PE dense: conv q -> conv k -> transposes ->
    sim pair0 -> v-conv (overlapping exp pair0 on ACT) -> sim pair1 ->
    AV pair0 -> AV pair1, with v inputs DMA'd last.
"""

import sys

sys.path.insert(0, "/opt/trn_rl_repo")

import ml_dtypes
import numpy as np

import concourse.bass as bass
import concourse.mybir as mybir
import concourse.tile as tile
from concourse.bass_utils import run_bass_kernel_spmd
from concourse.masks import make_identity

BF16 = mybir.dt.bfloat16
F32 = mybir.dt.float32
NPBF = ml_dtypes.bfloat16

B, N, D, WS = 32, 4096, 64, 128
W = N // WS              # 32 windows
NCORES = 8
BL = B // NCORES         # 4 batches per core
SCALE = D ** -0.5
ROPE_BASE = 10000.0

XCOLS = D * W * BL       # x / qc / kc cols: (d, w, b) = 8192
VCOLS = (D + 1) * W * BL  # vc cols: (d(+ones), w, b) = 8320
PCOLS = 2 * W * 2 * WS   # p cols per pair: (b2, m, 256) = 16384
PB = W * 2 * WS          # 8192: p cols per batch
CH_D = 16                # channels per input chunk
CH_COLS = CH_D * 2 * WS  # toeplitz-chunk cols (d, lo|hi, i) = 4096
XCH_COLS = CH_D * W * BL  # x-chunk cols (d, w, b) = 2048
NCH = D // CH_D          # 4 chunks per tensor


def _split_multiwaits(nc, max_waits=1):
    """walrus in this env rejects >1 sem wait per instruction; split extras
    into standalone NoOp waits inserted just before, on the same engine."""
    n_fixed = 0
    for fn in nc.m.functions:
        for bb in fn.blocks:
            insts = bb.instructions
            new_list = []
            changed = False
            for inst in insts:
                si = inst.sync_info
                if si is not None and si.on_wait and len(si.on_wait) > max_waits:
                    waits = list(si.on_wait)
                    for w in waits[:-max_waits]:
                        nop = mybir.InstNoOp(
                            name=f"{inst.name}-xw{n_fixed}",
                            engine=inst.engine,
                            ins=[],
                            outs=[],
                            sync_info=mybir.SyncInfo(on_wait=[w], on_update=[]),
                        )
                        new_list.append(nop)
                        n_fixed += 1
                    si.on_wait = waits[-max_waits:]
                    changed = True
                new_list.append(inst)
            if changed:
                bb.instructions = new_list
    return n_fixed


def _ap(t, offset, dims):
    """AP over tile t: partition dim kept, free dims replaced."""
    return bass.AP(tensor=t.tensor, offset=t.offset + offset, ap=[t.ap[0]] + dims)


def _ap_dram(t, offset, dims, row_elems):
    """AP over a [128, row_elems] dram tensor: partition stride row_elems."""
    return bass.AP(tensor=t, offset=offset, ap=[[row_elems, 128]] + dims)


def _build_program():
    nc = bass.Bass()
    # x: [128 j, (d, w, b)] host-pretransposed, fully contiguous
    xq = nc.dram_tensor("xq", [128, XCOLS], BF16, kind="ExternalInput")
    xk = nc.dram_tensor("xk", [128, XCOLS], BF16, kind="ExternalInput")
    xv = nc.dram_tensor("xv", [128, XCOLS], BF16, kind="ExternalInput")
    # toeplitz conv bands: [128 j, (d, lo|hi, i)] contiguous
    cq = nc.dram_tensor("cq", [128, D * 2 * WS], BF16, kind="ExternalInput")
    ck = nc.dram_tensor("ck", [128, D * 2 * WS], BF16, kind="ExternalInput")
    cv = nc.dram_tensor("cv", [128, D * 2 * WS], BF16, kind="ExternalInput")
    # rope tables: [128 i, (d, w, b2)] (replicated over the 2 pair batches)
    cosb = nc.dram_tensor("cosb", [128, D * W * 2], BF16, kind="ExternalInput")
    sinb = nc.dram_tensor("sinb", [128, D * W * 2], BF16, kind="ExternalInput")
    out = nc.dram_tensor("out", [BL, N, D], F32, kind="ExternalOutput")

    xdram = {"q": xq, "k": xk, "v": xv}
    cdram = {"q": cq, "k": ck, "v": cv}

    with tile.TileContext(nc) as tc:
        import contextlib

        with contextlib.ExitStack() as ctx:
            const = ctx.enter_context(tc.tile_pool(name="const", bufs=1))
            tabs = ctx.enter_context(tc.tile_pool(name="tabs", bufs=1))
            cpool = ctx.enter_context(tc.tile_pool(name="cw", bufs=5))
            xpool = ctx.enter_context(tc.tile_pool(name="x", bufs=5))
            rtmp = ctx.enter_context(tc.tile_pool(name="rt", bufs=1))
            big = ctx.enter_context(tc.tile_pool(name="big", bufs=2))
            vpool = ctx.enter_context(tc.tile_pool(name="v", bufs=1))
            qtp = ctx.enter_context(tc.tile_pool(name="qt", bufs=4))
            opool = ctx.enter_context(tc.tile_pool(name="o", bufs=2))
            spool = ctx.enter_context(tc.tile_pool(name="s", bufs=2))

            # ---- input DMAs, earliest-needed first, split across the two
            # HWDGE queues (sync + activation engine).
            costab = tabs.tile([128, D * W * 2], BF16)
            sintab = tabs.tile([128, D * W * 2], BF16)

            cch = {}
            xch = {}
            rr = [0]

            def q_alt():
                rr[0] += 1
                return nc.sync if rr[0] % 2 else nc.scalar

            def issue_inputs(name):
                for c in range(NCH):
                    ct = cpool.tile([128, CH_COLS], BF16, tag="cw",
                                    name=f"c_{name}{c}")
                    cch[(name, c)] = ct
                    q_alt().dma_start(
                        out=ct[:],
                        in_=_ap_dram(cdram[name], c * CH_COLS, [[1, CH_COLS]],
                                     D * 2 * WS))
                    xt = xpool.tile([128, XCH_COLS], BF16, tag="x",
                                    name=f"x_{name}{c}")
                    xch[(name, c)] = xt
                    q_alt().dma_start(
                        out=xt[:],
                        in_=_ap_dram(xdram[name], c * XCH_COLS, [[1, XCH_COLS]],
                                     XCOLS))

            issue_inputs("q")
            # rope tables are not needed until rope-q (~after conv q)
            nc.sync.dma_start(
                out=costab[:], in_=_ap_dram(cosb, 0, [[1, D * W * 2]], D * W * 2))
            nc.scalar.dma_start(
                out=sintab[:], in_=_ap_dram(sinb, 0, [[1, D * W * 2]], D * W * 2))
            issue_inputs("k")
            issue_inputs("v")

            # constants
            ident = const.tile([128, 128], BF16)
            make_identity(nc, ident)
            tri = const.tile([128, 128], BF16)  # tri[j,i] = 1 if i>=j else 0
            nc.vector.memset(tri[:], 1.0)
            nc.gpsimd.affine_select(
                out=tri[:], in_=tri[:], compare_op=mybir.AluOpType.is_ge,
                fill=0.0, base=0, channel_multiplier=-1, pattern=[[1, 128]],
            )

            # persistent activation tiles; qc+kc share one big buffer, p gets
            # a fresh one per pair (rotating back onto qc/kc space for pair 1)
            qckc = big.tile([128, 2 * XCOLS], BF16, tag="big", name="qckc")
            vc = vpool.tile([128, VCOLS], BF16)            # (d|ones, w, b)
            nc.vector.memset(vc[:, D * BL * W:], 1.0)      # ones column block

            drain_rr = [0]

            def drain(dst, src, act=True):
                # GpSimd cannot access PSUM; conv drains ride ACT so the
                # DVE's conv-phase is rope-only; transposes alternate
                if act:
                    nc.scalar.copy(dst, src)
                    return
                e = drain_rr[0] % 2
                drain_rr[0] += 1
                if e == 0:
                    nc.scalar.copy(dst, src)
                else:
                    nc.vector.tensor_copy(dst, src)

            def emit_rope(base, pair, eng):
                """in-place rope on (w, b2, d) views of qckc at col base."""
                po = base + pair * 2 * D  # col offset of batch pair
                t2 = rtmp.tile([128, W * 2 * D], BF16, tag="rt",
                               name=f"rt{pair}")
                # t2[d] = x[d^1] * sin[d]  (sin pre-signed by output parity);
                # partner swap via inner (u, e) dims reading col 2u + (1-e)
                part_in = _ap(qckc, po + 1,
                              [[4 * D, W], [D, 2], [2, D // 2], [-1, 2]])
                sin_in = _ap(sintab, 0,
                             [[2 * D, W], [D, 2], [2, D // 2], [1, 2]])
                t2v = _ap(t2, 0, [[2 * D, W], [D, 2], [2, D // 2], [1, 2]])
                eng.tensor_mul(t2v, part_in, sin_in)
                xv_ = _ap(qckc, po, [[4 * D, W], [1, 2 * D]])
                cos_in = _ap(costab, 0, [[2 * D, W], [1, 2 * D]])
                eng.tensor_mul(xv_, xv_, cos_in)
                eng.tensor_add(xv_, xv_, _ap(t2, 0, [[2 * D, W], [1, 2 * D]]))

            qT = {}  # (tensor, pair) -> [128 (b2,d), (w,i)] bf16

            def emit_transposes(name, pair, pool):
                base = 0 if name == "q" else XCOLS
                qt = qtp.tile([128, W * WS], BF16, tag="qt",
                              name=f"qt_{name}{pair}")
                for w4 in range(8):
                    tp = pool.tile([128, 512], F32, name=f"tp{pair}")
                    for wi in range(4):
                        w = w4 * 4 + wi
                        # stationary = (b2, d) cols of window w: contiguous
                        co = base + w * BL * D + pair * 2 * D
                        src = qckc[:, co: co + 2 * D]
                        nc.tensor.matmul(
                            tp[:, wi * 128: (wi + 1) * 128],
                            src, ident[:], start=True, stop=True,
                        )
                    drain(qt[:, w4 * 512: (w4 + 1) * 512], tp[:], act=False)
                qT[(name, pair)] = qt

            # ---- conv q, k via circulant + lower-band matmuls
            with tc.tile_pool(name="convps", bufs=2, space="PSUM") as convps, \
                 tc.tile_pool(name="tps", bufs=2, space="PSUM") as tps:
                def emit_conv_qk(name, c):
                    base = 0 if name == "q" else XCOLS
                    ct = cch[(name, c)]
                    xt = xch[(name, c)]
                    for g2 in range(2):  # 8 channels per psum group
                        # psum layout (h=dd//4, w, b, dd%4): each matmul's
                        # strided output stays inside one bank
                        cp = convps.tile([128, 8 * 128], F32)
                        for dd in range(8):
                            dl = g2 * 8 + dd   # channel within chunk
                            po = (dd // 4) * 512 + dd % 4
                            hi = ct[:, dl * 256 + 128: dl * 256 + 256]
                            lo = ct[:, dl * 256: dl * 256 + 128]
                            nc.tensor.matmul(
                                _ap(cp, po, [[16, W], [4, BL]]),
                                hi,
                                _ap(xt, dl * W * BL, [[1, W * BL]]),
                                start=True, stop=False)
                            # prev-window band: windows >= 1 only
                            nc.tensor.matmul(
                                _ap(cp, po + 16, [[16, W - 1], [4, BL]]),
                                lo,
                                _ap(xt, dl * W * BL, [[1, (W - 1) * BL]]),
                                start=False, stop=True,
                                skip_group_check=True)
                        d0 = base + (c * 2 + g2) * 8
                        dst = _ap(qckc, d0,
                                  [[4, 2], [BL * D, W], [D, BL], [1, 4]])
                        drain(dst, cp[:])

                for c in range(NCH):
                    emit_conv_qk("q", c)
                emit_conv_qk("k", 0)
                emit_rope(0, 0, nc.vector)
                emit_conv_qk("k", 1)
                emit_rope(0, 1, nc.vector)
                emit_conv_qk("k", 2)
                emit_conv_qk("k", 3)
                emit_rope(XCOLS, 0, nc.vector)

                # ---- transposes: q transposes overlap rope-k on DVE;
                # pair 0 of both tensors first so sim can start early
                emit_transposes("q", 0, tps)
                emit_rope(XCOLS, 1, nc.vector)
                emit_transposes("k", 0, tps)
                emit_transposes("q", 1, tps)
                emit_transposes("k", 1, tps)

            # ---- attention + v-conv
            with tc.tile_pool(name="convps2", bufs=2, space="PSUM") as convps2, \
                 tc.tile_pool(name="simps", bufs=2, space="PSUM") as simps, \
                 tc.tile_pool(name="avps", bufs=2, space="PSUM") as avps:

                pt = {}

                def emit_sim(pair):
                    qt = qT[("q", pair)]
                    kt = qT[("k", pair)]
                    p = big.tile([128, PCOLS], BF16, tag="big",
                                 name=f"p{pair}")
                    pt[pair] = p
                    # sim passes, 2 m's per group, both batches via row tiles
                    for g2 in range(16):
                        sp = simps.tile([128, 1024], F32)
                        m0 = g2 * 2
                        ncol_g = 0
                        for mi in range(2):
                            m = m0 + mi
                            ncols = 256 if m < W - 1 else 128
                            for h in range(2):  # batch half (row tile)
                                nc.tensor.matmul(
                                    sp[:, h * 512 + mi * 256:
                                       h * 512 + mi * 256 + ncols],
                                    kt[h * 64: h * 64 + 64,
                                       m * 128: (m + 1) * 128],
                                    qt[h * 64: h * 64 + 64,
                                       m * 128: m * 128 + ncols],
                                    start=True, stop=True,
                                )
                            ncol_g += ncols
                        # exp of both banks -> p[(b2, m0..m0+1, :)]
                        esrc = _ap(sp, 0, [[512, 2], [1, ncol_g]])
                        edst = _ap(p, m0 * 256, [[PB, 2], [1, ncol_g]])
                        nc.scalar.activation(
                            edst, esrc, mybir.ActivationFunctionType.Exp)

                def emit_mask(pair):
                    p = pt[pair]
                    # pad-row fixup (global key position 0 fully masked)
                    for b2 in range(2):
                        nc.vector.memset(p[0:1, b2 * PB: b2 * PB + 256], 0.0)
                    # causal mask: own-halves *= tri; per 4-m chunk,
                    # alternating DVE / GpSimd
                    for mc in range(8):
                        pview = _ap(p, mc * 4 * 256,
                                    [[PB, 2], [256, 4], [1, 128]])
                        tri_b = _ap(tri, 0, [[0, 2], [0, 4], [1, 128]])
                        eng = nc.gpsimd if mc % 2 else nc.vector
                        eng.tensor_mul(pview, pview, tri_b)
                    # all-masked-row fixup: query 0 attends uniformly
                    for b2 in range(2):
                        nc.vector.memset(p[:, b2 * PB: b2 * PB + 1], 1.0)

                def emit_vconv_part(part):
                    # v-conv in 4-channel psum groups (flat: vc is (d, w, b))
                    for c in range(2 * part, 2 * part + 2):
                        ct = cch[("v", c)]
                        xt = xch[("v", c)]
                        for g4 in range(4):  # 4 channels per psum group
                            cp = convps2.tile([128, 4 * 128], F32,
                                              name="vcp")
                            for dd in range(4):
                                dl = g4 * 4 + dd
                                ps = cp[:, dd * 128: (dd + 1) * 128]
                                ps_lo = cp[:, dd * 128 + BL: (dd + 1) * 128]
                                hi = ct[:, dl * 256 + 128: dl * 256 + 256]
                                lo = ct[:, dl * 256: dl * 256 + 128]
                                nc.tensor.matmul(
                                    ps, hi,
                                    _ap(xt, dl * W * BL, [[1, W * BL]]),
                                    start=True, stop=False)
                                nc.tensor.matmul(
                                    ps_lo, lo,
                                    _ap(xt, dl * W * BL,
                                        [[1, (W - 1) * BL]]),
                                    start=False, stop=True,
                                    skip_group_check=True)
                            dst = vc[:, (c * 4 + g4) * 512:
                                     (c * 4 + g4 + 1) * 512]
                            nc.vector.tensor_copy(dst, cp[:])

                def emit_av(pair, chunk, b2):
                    p = pt[pair]
                    b = pair * 2 + b2
                    pb = b2 * PB
                    sr = spool.tile([128, 8], F32, tag="sr")
                    if True:
                        w0 = chunk * 7
                        nwin = min(7, W - w0)
                        ot = opool.tile([128, 7 * D], F32, tag="ot")
                        av = avps.tile([128, 512], F32)
                        for k in range(nwin):
                            w = w0 + k
                            own = p[:, pb + w * 256: pb + w * 256 + 128]
                            ov = av[:, k * 65: k * 65 + 65]
                            vw = _ap(vc, w * BL + b, [[W * BL, D + 1]])
                            first_only = w == 0
                            nc.tensor.matmul(ov, own, vw, start=True,
                                             stop=first_only)
                            if w > 0:
                                prev = p[:, pb + (w - 1) * 256 + 128:
                                         pb + w * 256]
                                vprev = _ap(vc, (w - 1) * BL + b,
                                            [[W * BL, D + 1]])
                                nc.tensor.matmul(ov, prev, vprev,
                                                 start=False, stop=True,
                                                 skip_group_check=True)
                        if chunk == 0:
                            # window-0 query-0 sum correction (+128 pad)
                            nc.vector.tensor_scalar_add(
                                av[0:1, 64:65], av[0:1, 64:65], 128.0)
                        # normalize: recip of s, broadcast-mul
                        nc.vector.reciprocal(
                            sr[:, :nwin], _ap(av, 64, [[65, nwin]]))
                        avv = _ap(av, 0, [[65, nwin], [1, D]])
                        srv = _ap(sr, 0, [[1, nwin], [0, D]])
                        otv = _ap(ot, 0, [[D, nwin], [1, D]])
                        nc.vector.tensor_mul(otv, avv, srv)
                        dstd = bass.AP(
                            tensor=out, offset=b * N * D + w0 * WS * D,
                            ap=[[D, 128], [WS * D, nwin], [1, D]],
                        )
                        oeng = nc.scalar if (pair == 1 and b2 == 1) else nc.sync
                        oeng.dma_start(
                            out=dstd, in_=_ap(ot, 0, [[D, nwin], [1, D]]))

                emit_sim(0)
                emit_vconv_part(0)
                emit_vconv_part(1)
                emit_sim(1)
                emit_mask(0)
                for chunk in range(5):
                    emit_av(0, chunk, 0)
                    emit_av(0, chunk, 1)
                emit_mask(1)
                for chunk in range(5):
                    emit_av(1, chunk, 0)
                    emit_av(1, chunk, 1)

    _split_multiwaits(nc)
    return nc


_PROG = None


def _get_prog():
    global _PROG
    if _PROG is None:
        _PROG = _build_program()
    return _PROG


def _host_prep(q, k, v, wq, wk, wv):
    """Build per-core input maps (bf16 casts + layout transforms)."""
    jj = np.arange(WS)[:, None]
    ii = np.arange(WS)[None, :]
    lod = jj - ii - 1
    hid = jj - ii + (WS - 1)
    lom = (lod >= 0) & (lod < WS)
    him = (hid >= 0) & (hid < WS)
    lodc = np.clip(lod, 0, WS - 1)
    hidc = np.clip(hid, 0, WS - 1)

    def toep(w, scale=1.0):
        wd = np.asarray(w, np.float32).reshape(D, WS) * scale
        t = np.zeros((D, 2, WS, WS), np.float32)
        t[:, 0][:, lom] = wd[:, lodc[lom]]
        t[:, 1][:, him] = wd[:, hidc[him]]
        # [d, half, j, i] -> [j, (d, half, i)]
        return np.ascontiguousarray(t.transpose(2, 0, 1, 3)).reshape(
            WS, D * 2 * WS).astype(NPBF)

    cq_np = toep(wq, SCALE)
    ck_np = toep(wk)
    cv_np = toep(wv)

    theta = 1.0 / ROPE_BASE ** (np.arange(0, D, 2, dtype=np.float32) / D)
    pm = np.arange(N, dtype=np.float32)[:, None] * theta[None, :]
    cos = np.repeat(np.cos(pm), 2, axis=-1)  # [n, d]
    sin = np.repeat(np.sin(pm), 2, axis=-1)
    sgn = np.where(np.arange(D) % 2 == 0, -1.0, 1.0).astype(np.float32)

    # [j, (w, b2, d)] layout, replicated over the 2 batches of a pair
    def rope_table(t):
        tt = np.ascontiguousarray(t.reshape(W, WS, D).transpose(1, 0, 2))
        tt = np.repeat(tt.reshape(WS, W, 1, D), 2, axis=2)
        return np.ascontiguousarray(tt).reshape(WS, W * 2 * D).astype(NPBF)

    cosb_np = rope_table(cos)
    sinb_np = rope_table(sin * sgn[None, :])

    def xprep(x, sl):
        # [BL, N, D] -> [j, (d, w, b)] contiguous bf16
        xb = np.asarray(x[sl], np.float32).reshape(BL, W, WS, D)
        return np.ascontiguousarray(xb.transpose(2, 3, 1, 0)).reshape(
            WS, D * W * BL).astype(NPBF)

    in_maps = []
    for c in range(NCORES):
        sl = slice(c * BL, (c + 1) * BL)
        in_maps.append({
            "xq": xprep(q, sl),
            "xk": xprep(k, sl),
            "xv": xprep(v, sl),
            "cq": cq_np, "ck": ck_np, "cv": cv_np,
            "cosb": cosb_np, "sinb": sinb_np,
        })
    return in_maps


def _install_ntff_hook():
    """Provide antenv.axon_hooks with a ctypes NTFF profile hook (the slim
    container lacks it); enables trace=True under axon."""
    import sys as _sys
    import types
    import ctypes
    import contextlib

    try:
        from antenv.axon_hooks import get_axon_ntff_profile_hook  # noqa: F401
        return
    except ImportError:
        pass
    so_path = "/opt/axon/libaxon_pjrt.so"
    try:
        lib = ctypes.CDLL(so_path)
    except OSError:
        return
    if not hasattr(lib, "axon_start_nrt_profile"):
        return
    lib.axon_start_nrt_profile.argtypes = [
        ctypes.POINTER(ctypes.c_int64), ctypes.c_size_t]
    lib.axon_start_nrt_profile.restype = ctypes.c_int64
    lib.axon_stop_nrt_profile.argtypes = [ctypes.c_char_p]
    lib.axon_stop_nrt_profile.restype = ctypes.c_int64

    @contextlib.contextmanager
    def _hook(output_dir, device_ids):
        import jax
        jax.devices()
        if device_ids:
            ids = (ctypes.c_int64 * len(device_ids))(*device_ids)
            rc = lib.axon_start_nrt_profile(ids, len(device_ids))
        else:
            rc = lib.axon_start_nrt_profile(None, 0)
        if rc != 0:
            raise RuntimeError(f"axon_start_nrt_profile rc={rc}")
        try:
            yield
        finally:
            n = lib.axon_stop_nrt_profile(str(output_dir).encode())
            print(f"profile: {n} file(s) written to {output_dir}")

    import antenv

    mod = types.ModuleType("antenv.axon_hooks")
    _state = {"hook": _hook}
    mod.set_axon_ntff_profile_hook = lambda h: _state.__setitem__("hook", h)
    mod.get_axon_ntff_profile_hook = lambda: _state["hook"]
    _sys.modules["antenv.axon_hooks"] = mod
    antenv.axon_hooks = mod


def run(q, k, v, wq, wk, wv, trace=False):
    nc = _get_prog()
    in_maps = _host_prep(q, k, v, wq, wk, wv)
    if trace:
        _install_ntff_hook()
    res = run_bass_kernel_spmd(nc, in_maps, core_ids=list(range(NCORES)),
                               trace=trace)
    outp = np.concatenate([res.results[c]["out"] for c in range(NCORES)], axis=0)
    return outp, res


def kernel(q, k, v, wq, wk, wv):
    outp, _ = run(q, k, v, wq, wk, wv)
    return outp
